# revision 1
# baseline (speedup 1.0000x reference)
"""CosSim2D (3x3, same-pad) Trainium2 kernel, 8-core batch-parallel.

Layout strategy per core (one 224x224x32 image):
  - Host pads image to 226x226 and flattens to xp[p, c] (p = y*226+x), bf16.
  - Device: natural-layout chunks are block-transposed (DVE 32x32) into
    channel-on-partition strips T[32c, px] -- 4 independent segments on the
    4 partition groups so every elementwise pass runs 128 partitions wide.
  - conv: 9 matmuls per 448-px chunk (K=32 c, M=32 f), tap shifts applied as
    free-dim offsets on the rhs AP; 4 chunks (one per segment / row-group /
    col-group) accumulate concurrently into one [128, 448] PSUM tile.
  - norm: sq = Square(T); 3x3 box pre-sum on DVE; one extra matmul with an
    all-ones [32,32] lhsT fills a second PSUM tile with sum_c(boxsq) rows.
  - Evac: DVE StreamTranspose [128,448] PSUM->SBUF gives [px-in-block, f]
    blocks; the norm tile comes out broadcast along f for free.
  - sim = conv * 1/(sqrt(ns)+qt) on strided/compact tiles; bf16 store in a
    blocked scratch layout; host un-blocks, applies sign*(|x|+eps)^e, casts.
"""

import numpy as np

import concourse.bass as bass
import concourse.mybir as mybir
import concourse.tile as tile
from concourse import bacc
from concourse.bass_utils import run_bass_kernel_spmd

K = 3
EPS = 1e-12
H = W = 224
C = 32
F = 32
B = 8
XP = 226                 # padded row stride
P_NEED = 223 * 226 + 224  # exclusive max base-p actually used (50622)

CH = 448                 # px per chunk (= matmul N)
CPS = 8                  # chunks per segment per band
SEGS = 4
BANDS = 4
CHUNKS = BANDS * SEGS * CPS          # 128 chunks >= ceil(50622/448)=113
STRIP = CPS * CH + 2 * XP + 2 + 446  # per-(band,seg) strip px incl. halo
STRIP = ((STRIP + 31) // 32) * 32    # 32-divisible for block transpose
XPN = (BANDS * SEGS * CPS) * CH + STRIP  # padded xp length (safe upper bound)
JB = STRIP // 32         # 32-px blocks per strip

_compiled = None
TRACE = False
LAST_PROFILE = None


def _build(qtv: float):
    nc = bacc.Bacc()
    f32 = mybir.dt.float32
    bf16 = mybir.dt.bfloat16

    xp = nc.declare_dram_parameter("xp", [XPN * C], bf16, isOutput=False)
    wt = nc.declare_dram_parameter("wt", [9 * C * F], bf16, isOutput=False)
    odev = nc.declare_dram_parameter(
        "odev", [CHUNKS // 4, 128, CH], bf16, isOutput=True
    )

    with tile.TileContext(nc) as tc:
        with (
            tc.tile_pool(name="consts", bufs=1) as consts,
            tc.tile_pool(name="band", bufs=2) as band_pool,
            tc.tile_pool(name="round", bufs=3) as round_pool,
            tc.tile_pool(name="psum", bufs=4, space="PSUM") as psum_pool,
        ):
            # ---- constants ----
            # weights: 9 taps of [32c, 32f]
            # weight/ones stationaries replicated on all 4 partition groups:
            # walrus requires lhsT and rhs to share the SBUF base partition.
            wts = consts.tile([128, 9 * F], bf16, tag="wts")
            for g in range(SEGS):
                nc.sync.dma_start(
                    out=wts[32 * g : 32 * g + 32, :],
                    in_=wt.rearrange("(c tf) -> c tf", c=C),
                )
            ones_lhs = consts.tile([128, F], bf16, tag="ones")
            nc.vector.memset(ones_lhs, 1.0)

            xp2d = xp.rearrange("(p c) -> p c", c=C)

            for b in range(BANDS):
                # ---- per-band prep: load 4 segment strips, transpose, square,
                #      3x3 box-sum of squares ----
                L = band_pool.tile([128, JB * 32], bf16, tag="L")
                for g in range(SEGS):
                    p0 = (b * SEGS * CPS + g * CPS) * CH
                    src = xp2d[p0 : p0 + STRIP, :].rearrange(
                        "(j i) c -> i j c", i=32
                    )
                    dst = L[32 * g : 32 * g + 32, :].rearrange(
                        "i (j c) -> i j c", c=C
                    )
                    nc.sync.dma_start(out=dst, in_=src)

                # Absorb the 4 DMA waits into tiny same-engine copies: the
                # StreamTranspose struct has too few sync-wait slots for 4.
                sink = band_pool.tile([128, 1], bf16, tag="sink")
                for g in range(SEGS):
                    nc.vector.tensor_copy(
                        sink[32 * g : 32 * g + 32, :],
                        L[32 * g : 32 * g + 32, 0:1],
                    )
                T = band_pool.tile([128, JB * 32], bf16, tag="T")
                nc.vector.transpose(out=T, in_=L)

                SQ = band_pool.tile([128, JB * 32], bf16, tag="SQ")
                nc.scalar.activation(
                    SQ, T, mybir.ActivationFunctionType.Square
                )
                # horizontal (dx) then vertical (dy) box pre-sum, bf16 2x TT
                SQH = band_pool.tile([128, JB * 32], bf16, tag="SQH")
                n_h = JB * 32 - 2
                nc.vector.tensor_add(SQH[:, :n_h], SQ[:, :n_h], SQ[:, 1 : 1 + n_h])
                nc.vector.tensor_add(SQH[:, :n_h], SQH[:, :n_h], SQ[:, 2 : 2 + n_h])
                SQB = band_pool.tile([128, JB * 32], bf16, tag="SQB")
                n_v = JB * 32 - 2 * XP
                nc.vector.tensor_add(
                    SQB[:, :n_v], SQH[:, :n_v], SQH[:, XP : XP + n_v]
                )
                nc.vector.tensor_add(
                    SQB[:, :n_v], SQB[:, :n_v], SQH[:, 2 * XP : 2 * XP + n_v]
                )

                for r in range(CPS):
                    # ---- 4 concurrent chunks (one per segment) ----
                    P1 = psum_pool.tile([128, CH], f32, tag="P1")
                    P2 = psum_pool.tile([128, CH], f32, tag="P2")
                    for g in range(SEGS):
                        gp = 32 * g
                        loc = r * CH
                        for t in range(9):
                            dy, dx = t // 3, t % 3
                            off = loc + dy * XP + dx
                            nc.tensor.matmul(
                                P1[gp : gp + 32, :],
                                wts[gp : gp + 32, t * F : (t + 1) * F],
                                T[gp : gp + 32, off : off + CH],
                                start=(t == 0),
                                stop=(t == 8),
                                tile_position=(gp, gp),
                            )
                        nc.tensor.matmul(
                            P2[gp : gp + 32, :],
                            ones_lhs[gp : gp + 32, :],
                            SQB[gp : gp + 32, loc : loc + CH],
                            start=True,
                            stop=True,
                            tile_position=(gp, gp),
                        )

                    # ---- evac + transpose (px onto partitions) ----
                    CT = round_pool.tile([128, CH], f32, tag="CT")
                    nc.vector.transpose(out=CT, in_=P1)
                    NB = round_pool.tile([128, CH], f32, tag="NB")
                    nc.vector.transpose(out=NB, in_=P2)

                    # ---- normalization ----
                    # NB[p, 32j+*] = ns(px) broadcast along f already.
                    nsj = NB[:, 0 : CH : 32]            # [128, 14] strided
                    XNQ = round_pool.tile([128, 16], f32, tag="XNQ")
                    nc.scalar.activation(
                        XNQ[:, : CH // 32], nsj,
                        mybir.ActivationFunctionType.Sqrt,
                    )
                    nc.scalar.add(XNQ[:, : CH // 32], XNQ[:, : CH // 32], qtv)
                    INV = round_pool.tile([128, 16], f32, tag="INV")
                    nc.vector.reciprocal(INV[:, : CH // 32], XNQ[:, : CH // 32])

                    SIM = round_pool.tile([128, CH], bf16, tag="SIM")
                    inv_b = INV[:, : CH // 32].rearrange(
                        "p (j one) -> p j one", one=1
                    )
                    nc.vector.tensor_mul(
                        SIM.rearrange("p (j f) -> p j f", f=32),
                        CT.rearrange("p (j f) -> p j f", f=32),
                        inv_b.to_broadcast((128, CH // 32, 32)),
                    )

                    ridx = b * CPS + r
                    nc.sync.dma_start(out=odev[ridx, :, :], in_=SIM)

    nc.compile()
    return nc


def _host_pack(image_b, w, q):
    """Per-core input prep: pad+flatten image (bf16), normalized weights."""
    qtv = np.float32(np.float32(q[0]) * np.float32(q[0]) / np.float32(10.0))
    w0 = w[0].astype(np.float32)  # [288, 32]
    wn = np.sqrt(np.maximum((w0 * w0).sum(axis=0), np.float32(EPS))) + qtv
    wnorm = (w0 / wn[None, :]).astype(np.float32)
    import ml_dtypes

    # reference im2col order: (dy*3+dx)*C + c. Device wants [c, (t f)].
    wt_bf = np.ascontiguousarray(
        wnorm.reshape(9, C, F).transpose(1, 0, 2)
    ).astype(ml_dtypes.bfloat16).reshape(-1)

    xp_full = np.zeros((XPN, C), dtype=ml_dtypes.bfloat16)
    padded = np.zeros((XP, XP, C), dtype=np.float32)
    padded[1:225, 1:225, :] = image_b
    xp_full[: XP * XP] = padded.reshape(XP * XP, C).astype(ml_dtypes.bfloat16)
    return xp_full.reshape(-1), wt_bf, float(qtv)


def _host_unpack(odev_b):
    """odev [CHUNKS//4, 128, 448] bf16 -> sim over xp-base-p index."""
    # R = band*CPS + r ; partition = 32g + a ; col = 32j + bfree
    arr = np.asarray(odev_b, dtype=np.float32)
    arr = arr.reshape(BANDS, CPS, SEGS, 32, CH // 32, 32)
    # chunk index c = band*32 + g*8 + r ; px = c*448 + 32j + a ; f = bfree
    arr = arr.transpose(0, 2, 1, 4, 3, 5)  # band, g, r, j, a, f
    sim_p = arr.reshape(CHUNKS * CH, F)
    return sim_p


_PMAP = None


def _pmap():
    global _PMAP
    if _PMAP is None:
        y, x = np.mgrid[0:H, 0:W]
        _PMAP = (y * XP + x).reshape(-1)
    return _PMAP


def kernel(image, w, p, q):
    global _compiled
    image = np.asarray(image)
    w = np.asarray(w, dtype=np.float32)
    p = np.asarray(p, dtype=np.float32)
    q = np.asarray(q, dtype=np.float32)

    in_maps = []
    qtv = None
    for b in range(B):
        xpb, wtb, qtv = _host_pack(image[b].astype(np.float32), w, q)
        in_maps.append({"xp": xpb, "wt": wtb})

    if _compiled is None or _compiled[0] != qtv:
        _compiled = (qtv, _build(qtv))
    nc = _compiled[1]

    global LAST_PROFILE
    res = run_bass_kernel_spmd(
        nc, in_maps, core_ids=list(range(B)), trace=TRACE
    )
    LAST_PROFILE = res
    if TRACE and res.exec_time_ns is not None:
        print(f"HW exec time: {res.exec_time_ns} ns")

    e = (p * p) / np.float32(100.0)  # per-filter exponent
    out = np.empty((B, H * W, F), dtype=np.float32)
    pm = _pmap()
    for b in range(B):
        sim = _host_unpack(res.results[b]["odev"])[pm]  # [H*W, F] fp32
        out[b] = np.sign(sim) * np.power(np.abs(sim) + np.float32(EPS), e[None, :])
    return out.reshape(B, H, W, F)



# revision 2
# speedup vs baseline: 1.4401x; 1.4401x over previous
"""CosSim2D (3x3, same-pad) Trainium2 kernel, 8-core batch-parallel.

v2 layout strategy per core (one 224x224x32 image):
  - Host pads image to 226x226 and provides it CHANNEL-MAJOR as
    xpT[c, p] (p = y*226+x), bf16 -- no on-device transposes at all.
  - Device: each 3584-px strip is loaded 3x into a [96, STRIP] tile
    (partition group dy = strip shifted by dy*226), so each conv matmul
    contracts K=96 = 3 dy-taps x 32 channels; the 3 dx taps are free-dim
    offsets -> 3 matmuls per 512-px chunk instead of 9.
  - norm: a packed [128, STRIP] center tile (4 strips on partition
    groups) -> Square (scalar), 3x3 box pre-sum (gpsimd + vector), one
    ones-lhsT K=32 matmul per chunk fills P2 rows with sum-sq, already
    broadcast across its 32 output rows.
  - Evac without transposes: XNQ = sqrt(P2)+qt (scalar), INV (vector
    reciprocal), SIM = P1 * INV (vector) in [f, px] layout, bf16 DMA
    out; host un-permutes and applies sign*(|x|+eps)^e.
  - Last band is ragged: only 99 chunks (covering the 50622 used px)
    are computed instead of 128.
"""

import numpy as np

import concourse.bass as bass
import concourse.mybir as mybir
import concourse.tile as tile
from concourse import bacc
from concourse.bass_utils import run_bass_kernel_spmd

K = 3
EPS = 1e-12
H = W = 224
C = 32
F = 32
B = 8
XP = 226                  # padded row stride
P_NEED = 223 * 226 + 224  # exclusive max base-p actually used (50622)

CH = 512                  # px per chunk (= matmul N, one PSUM bank)
CPS = 7                   # chunks per strip
SPX = CPS * CH            # strip px span (3584)
NSTRIP = 15               # strips 0..14 used; strip 14 only 1 chunk
STRIP = SPX + 2 * XP + 2  # loaded strip length incl. halo (4038)
STRIP = ((STRIP + 7) // 8) * 8  # 4040
XPN = 54784               # padded xpT length (>= 14*3584+2*226+4040)
BANDS = 4
ROUNDS = CPS              # 7 rounds per band
N1 = STRIP - 2            # horizontal box-sum valid cols
N2 = STRIP - 2 * XP       # full box-sum valid cols (3588)


def _nch(s):
    if s <= 13:
        return CPS
    if s == 14:
        return 1
    return 0


_compiled = None
TRACE = False
LAST_PROFILE = None


def _build(qtv: float):
    nc = bacc.Bacc()
    f32 = mybir.dt.float32
    bf16 = mybir.dt.bfloat16

    xp = nc.declare_dram_parameter("xp", [C * XPN], bf16, isOutput=False)
    wt = nc.declare_dram_parameter("wt", [96 * 96], bf16, isOutput=False)
    odev = nc.declare_dram_parameter(
        "odev", [BANDS * ROUNDS, 128, CH], bf16, isOutput=True
    )

    xp2d = xp.rearrange("(c x) -> c x", c=C)

    with tile.TileContext(nc) as tc:
        with (
            tc.tile_pool(name="consts", bufs=1) as consts,
            tc.tile_pool(name="band", bufs=2) as band_pool,
            tc.tile_pool(name="round", bufs=3) as round_pool,
            tc.tile_pool(name="psum", bufs=4, space="PSUM") as psum_pool,
        ):
            # weights: [96, 96]: row 32*dy+c, col dx*F+f
            wts = consts.tile([96, 3 * F], bf16, tag="wts")
            nc.sync.dma_start(out=wts, in_=wt.rearrange("(k m) -> k m", m=3 * F))
            ones_lhs = consts.tile([128, F], bf16, tag="ones")
            nc.vector.memset(ones_lhs, 1.0)

            for b in range(BANDS):
                glist = [g for g in range(4) if _nch(4 * b + g) > 0]
                R = 32 * len(glist)

                # ---- per-band loads ----
                # center-copy tile packs the band's strips on partition
                # groups: feeds the squared-norm chain at full width.
                TC = band_pool.tile([128, STRIP], bf16, tag="TC")
                for g in glist:
                    p0 = (4 * b + g) * SPX
                    nc.sync.dma_start(
                        out=TC[32 * g : 32 * g + 32, :],
                        in_=xp2d[:, p0 : p0 + STRIP],
                    )
                # dy-stacked conv tiles, one per strip
                TD = []
                for g in range(4):
                    if g not in glist:
                        TD.append(None)
                        continue
                    t = band_pool.tile([96, STRIP], bf16, tag=f"TD{g}")
                    p0 = (4 * b + g) * SPX
                    for dy in range(3):
                        nc.sync.dma_start(
                            out=t[32 * dy : 32 * dy + 32, :],
                            in_=xp2d[:, p0 + dy * XP : p0 + dy * XP + STRIP],
                        )
                    TD.append(t)

                # ---- squared-norm chain ----
                SQ = band_pool.tile([128, STRIP], bf16, tag="SQ")
                nc.scalar.activation(
                    SQ[:R, :], TC[:R, :], mybir.ActivationFunctionType.Square
                )
                SQH = band_pool.tile([128, STRIP], bf16, tag="SQH")
                nc.gpsimd.tensor_add(
                    SQH[:R, :N1], SQ[:R, :N1], SQ[:R, 1 : 1 + N1]
                )
                nc.gpsimd.tensor_add(
                    SQH[:R, :N1], SQH[:R, :N1], SQ[:R, 2 : 2 + N1]
                )
                SQB = band_pool.tile([128, STRIP], bf16, tag="SQB")
                nc.vector.tensor_add(
                    SQB[:R, :N2], SQH[:R, :N2], SQH[:R, XP : XP + N2]
                )
                nc.vector.tensor_add(
                    SQB[:R, :N2], SQB[:R, :N2], SQH[:R, 2 * XP : 2 * XP + N2]
                )

                for r in range(ROUNDS):
                    ga = [g for g in glist if r < _nch(4 * b + g)]
                    Rr = 32 * len(ga)
                    P1 = psum_pool.tile([128, CH], f32, tag="P1")
                    P2 = psum_pool.tile([128, CH], f32, tag="P2")
                    loc = r * CH
                    for g in ga:
                        gp = 32 * g
                        for dx in range(3):
                            nc.tensor.matmul(
                                P1[gp : gp + 32, :],
                                wts[:, dx * F : (dx + 1) * F],
                                TD[g][:, loc + dx : loc + dx + CH],
                                start=(dx == 0),
                                stop=(dx == 2),
                                tile_position=(0, gp),
                            )
                        nc.tensor.matmul(
                            P2[gp : gp + 32, :],
                            ones_lhs[gp : gp + 32, :],
                            SQB[gp : gp + 32, loc : loc + CH],
                            start=True,
                            stop=True,
                            tile_position=(gp, gp),
                        )

                    # ---- evac: no transposes; P2 rows are already the
                    # per-px norms broadcast along f within each group ----
                    XNQ = round_pool.tile([128, CH], f32, tag="XNQ")
                    nc.scalar.activation(
                        XNQ[:Rr, :], P2[:Rr, :],
                        mybir.ActivationFunctionType.Sqrt,
                    )
                    nc.scalar.add(XNQ[:Rr, :], XNQ[:Rr, :], qtv)
                    INV = round_pool.tile([128, CH], f32, tag="INV")
                    nc.vector.reciprocal(INV[:Rr, :], XNQ[:Rr, :])
                    SIM = round_pool.tile([128, CH], bf16, tag="SIM")
                    nc.vector.tensor_mul(SIM[:Rr, :], P1[:Rr, :], INV[:Rr, :])

                    nc.sync.dma_start(
                        out=odev[b * ROUNDS + r, :Rr, :], in_=SIM[:Rr, :]
                    )

    nc.compile()
    return nc


def _host_pack(image_b, w, q):
    """Per-core input prep: channel-major padded image (bf16), packed
    normalized weights."""
    import ml_dtypes

    qtv = np.float32(np.float32(q[0]) * np.float32(q[0]) / np.float32(10.0))
    w0 = w[0].astype(np.float32)  # [288, 32]
    wn = np.sqrt(np.maximum((w0 * w0).sum(axis=0), np.float32(EPS))) + qtv
    wnorm = (w0 / wn[None, :]).astype(np.float32)
    # reference im2col order: (dy*3+dx)*C + c -> rows (dy,c), cols (dx,f)
    wt_bf = np.ascontiguousarray(
        wnorm.reshape(3, 3, C, F).transpose(0, 2, 1, 3)
    ).astype(ml_dtypes.bfloat16).reshape(-1)

    padded = np.zeros((XP, XP, C), dtype=np.float32)
    padded[1:225, 1:225, :] = image_b
    xpT = np.zeros((C, XPN), dtype=ml_dtypes.bfloat16)
    xpT[:, : XP * XP] = (
        padded.reshape(XP * XP, C).T.astype(ml_dtypes.bfloat16)
    )
    return xpT.reshape(-1), wt_bf, float(qtv)


def _host_unpack(odev_b):
    """odev [28, 128, 512] bf16 -> sim over xp-base-p index."""
    arr = np.asarray(odev_b, dtype=np.float32)
    arr = arr.reshape(BANDS, ROUNDS, 4, F, CH)
    arr = arr.transpose(0, 2, 1, 4, 3)  # b, g, r, n, f
    return arr.reshape(BANDS * 4 * ROUNDS * CH, F)


_PMAP = None


def _pmap():
    global _PMAP
    if _PMAP is None:
        y, x = np.mgrid[0:H, 0:W]
        _PMAP = (y * XP + x).reshape(-1)
    return _PMAP


def kernel(image, w, p, q):
    global _compiled
    image = np.asarray(image)
    w = np.asarray(w, dtype=np.float32)
    p = np.asarray(p, dtype=np.float32)
    q = np.asarray(q, dtype=np.float32)

    in_maps = []
    qtv = None
    for b in range(B):
        xpb, wtb, qtv = _host_pack(image[b].astype(np.float32), w, q)
        in_maps.append({"xp": xpb, "wt": wtb})

    if _compiled is None or _compiled[0] != qtv:
        _compiled = (qtv, _build(qtv))
    nc = _compiled[1]

    global LAST_PROFILE
    res = run_bass_kernel_spmd(
        nc, in_maps, core_ids=list(range(B)), trace=TRACE
    )
    LAST_PROFILE = res
    if TRACE and res.exec_time_ns is not None:
        print(f"HW exec time: {res.exec_time_ns} ns")

    e = (p * p) / np.float32(100.0)  # per-filter exponent
    out = np.empty((B, H * W, F), dtype=np.float32)
    pm = _pmap()
    for b in range(B):
        sim = _host_unpack(res.results[b]["odev"])[pm]  # [H*W, F] fp32
        out[b] = np.sign(sim) * np.power(np.abs(sim) + np.float32(EPS), e[None, :])
    return out.reshape(B, H, W, F)


# revision 3
# speedup vs baseline: 2.6768x; 1.8588x over previous
"""CosSim2D (3x3, same-pad) Trainium2 kernel, 8-core batch-parallel.

v3 layout strategy per core (one 224x224x32 image):
  - Host pads image to 226x226 and provides it CHANNEL-MAJOR as
    xpT[c, p] (p = y*226+x), bf16 -- no on-device transposes at all.
  - Device: each 3584-px strip is loaded 3x into a [96, STRIP] tile
    (partition group dy = strip shifted by dy*226), so each conv matmul
    contracts K=96 = 3 dy-taps x 32 channels; the 3 dx taps are free-dim
    offsets -> 3 matmuls per 512-px chunk instead of 9.
  - norm: a packed [128, STRIP] center tile (4 strips on partition
    groups) -> Square (scalar), 3x3 box pre-sum (vector), one ones-lhsT
    K=32 matmul per chunk fills P2 rows with sum-sq, already broadcast
    across its 32 output rows.
  - Evac without transposes: XNQ = sqrt(P2)+qt (scalar), INV (fast
    approx reciprocal), SIM = P1 * INV (vector) in [f, px] layout, bf16
    DMA out; host un-permutes and applies sign*(|x|+eps)^e.
  - Band b+1's loads and norm chain are emitted BEFORE band b's rounds
    so every engine queue prefetches one band ahead.
  - Last band is ragged: only 99 chunks (covering the 50622 used px).
"""

import numpy as np

import concourse.bass as bass
import concourse.mybir as mybir
import concourse.tile as tile
from concourse import bacc
from concourse.bass_utils import run_bass_kernel_spmd

K = 3
EPS = 1e-12
H = W = 224
C = 32
F = 32
B = 8
XP = 226                  # padded row stride
P_NEED = 223 * 226 + 224  # exclusive max base-p actually used (50622)

CH = 512                  # px per chunk (= matmul N, one PSUM bank)
CPS = 7                   # chunks per strip
SPX = CPS * CH            # strip px span (3584)
STRIP = SPX + 2 * XP + 2  # loaded strip length incl. halo (4038)
STRIP = ((STRIP + 7) // 8) * 8  # 4040
XPN = 54784               # padded xpT length (>= 14*3584+2*226+4040)
BANDS = 4
ROUNDS = CPS              # 7 rounds per band
N1 = STRIP - 2            # horizontal box-sum valid cols
N2 = STRIP - 2 * XP       # full box-sum valid cols (3588)


def _nch(s):
    if s <= 13:
        return CPS
    if s == 14:
        return 1
    return 0


_compiled = None
TRACE = False
LAST_PROFILE = None


def _build(qtv: float):
    nc = bacc.Bacc()
    f32 = mybir.dt.float32
    bf16 = mybir.dt.bfloat16

    xp = nc.declare_dram_parameter("xp", [C * XPN], bf16, isOutput=False)
    wt = nc.declare_dram_parameter("wt", [96 * 96], bf16, isOutput=False)
    odev = nc.declare_dram_parameter(
        "odev", [BANDS * ROUNDS, 128, CH], bf16, isOutput=True
    )

    xp2d = xp.rearrange("(c x) -> c x", c=C)

    with tile.TileContext(nc) as tc:
        with (
            tc.tile_pool(name="consts", bufs=1) as consts,
            tc.tile_pool(name="band", bufs=2) as band_pool,
            tc.tile_pool(name="round", bufs=3) as round_pool,
            tc.tile_pool(name="psum", bufs=4, space="PSUM") as psum_pool,
        ):
            # weights: [96, 96]: row 32*dy+c, col dx*F+f
            wts = consts.tile([96, 3 * F], bf16, tag="wts")
            nc.sync.dma_start(out=wts, in_=wt.rearrange("(k m) -> k m", m=3 * F))
            ones_lhs = consts.tile([128, F], bf16, tag="ones")
            nc.vector.memset(ones_lhs, 1.0)

            def emit_loads(b):
                glist = [g for g in range(4) if _nch(4 * b + g) > 0]
                TC = band_pool.tile([128, STRIP], bf16, tag="TC")
                for g in glist:
                    p0 = (4 * b + g) * SPX
                    nc.sync.dma_start(
                        out=TC[32 * g : 32 * g + 32, :],
                        in_=xp2d[:, p0 : p0 + STRIP],
                    )
                TD = []
                for g in range(4):
                    if g not in glist:
                        TD.append(None)
                        continue
                    t = band_pool.tile([96, STRIP], bf16, tag=f"TD{g}")
                    p0 = (4 * b + g) * SPX
                    for dy in range(3):
                        nc.sync.dma_start(
                            out=t[32 * dy : 32 * dy + 32, :],
                            in_=xp2d[:, p0 + dy * XP : p0 + dy * XP + STRIP],
                        )
                    TD.append(t)
                return glist, TC, TD

            def emit_sqchain(b, tiles):
                glist, TC, TD = tiles
                R = 32 * len(glist)
                SQ = band_pool.tile([128, STRIP], bf16, tag="SQ")
                nc.scalar.activation(
                    SQ[:R, :], TC[:R, :], mybir.ActivationFunctionType.Square
                )
                SQH = band_pool.tile([128, STRIP], bf16, tag="SQH")
                nc.vector.tensor_add(
                    SQH[:R, :N1], SQ[:R, :N1], SQ[:R, 1 : 1 + N1]
                )
                nc.vector.tensor_add(
                    SQH[:R, :N1], SQH[:R, :N1], SQ[:R, 2 : 2 + N1]
                )
                SQB = band_pool.tile([128, STRIP], bf16, tag="SQB")
                nc.vector.tensor_add(
                    SQB[:R, :N2], SQH[:R, :N2], SQH[:R, XP : XP + N2]
                )
                nc.vector.tensor_add(
                    SQB[:R, :N2], SQB[:R, :N2], SQH[:R, 2 * XP : 2 * XP + N2]
                )
                return SQB

            def emit_rounds(b, tiles, SQB):
                glist, TC, TD = tiles
                for r in range(ROUNDS):
                    ga = [g for g in glist if r < _nch(4 * b + g)]
                    Rr = 32 * len(ga)
                    P1 = psum_pool.tile([128, CH], f32, tag="P1")
                    P2 = psum_pool.tile([128, CH], f32, tag="P2")
                    loc = r * CH
                    for g in ga:
                        gp = 32 * g
                        for dx in range(3):
                            nc.tensor.matmul(
                                P1[gp : gp + 32, :],
                                wts[:, dx * F : (dx + 1) * F],
                                TD[g][:, loc + dx : loc + dx + CH],
                                start=(dx == 0),
                                stop=(dx == 2),
                                tile_position=(0, gp),
                            )
                        nc.tensor.matmul(
                            P2[gp : gp + 32, :],
                            ones_lhs[gp : gp + 32, :],
                            SQB[gp : gp + 32, loc : loc + CH],
                            start=True,
                            stop=True,
                            tile_position=(gp, gp),
                        )

                    # evac: no transposes; P2 rows are already the per-px
                    # norms broadcast along f within each group
                    XNQ = round_pool.tile([128, CH], f32, tag="XNQ")
                    nc.scalar.activation(
                        XNQ[:Rr, :], P2[:Rr, :],
                        mybir.ActivationFunctionType.Sqrt,
                    )
                    nc.scalar.add(XNQ[:Rr, :], XNQ[:Rr, :], qtv)
                    INV = round_pool.tile([128, CH], f32, tag="INV")
                    nc.vector.reciprocal_approx_fast(
                        out=INV[:Rr, :], in_=XNQ[:Rr, :]
                    )
                    SIM = round_pool.tile([128, CH], bf16, tag="SIM")
                    nc.vector.tensor_mul(SIM[:Rr, :], P1[:Rr, :], INV[:Rr, :])

                    nc.sync.dma_start(
                        out=odev[b * ROUNDS + r, :Rr, :], in_=SIM[:Rr, :]
                    )

            tiles_cur = emit_loads(0)
            sqb_cur = emit_sqchain(0, tiles_cur)
            for b in range(BANDS):
                tiles_next = sqb_next = None
                if b + 1 < BANDS:
                    tiles_next = emit_loads(b + 1)
                    sqb_next = emit_sqchain(b + 1, tiles_next)
                emit_rounds(b, tiles_cur, sqb_cur)
                tiles_cur, sqb_cur = tiles_next, sqb_next

    nc.compile()
    return nc


def _host_pack(image_b, w, q):
    """Per-core input prep: channel-major padded image (bf16), packed
    normalized weights."""
    import ml_dtypes

    qtv = np.float32(np.float32(q[0]) * np.float32(q[0]) / np.float32(10.0))
    w0 = w[0].astype(np.float32)  # [288, 32]
    wn = np.sqrt(np.maximum((w0 * w0).sum(axis=0), np.float32(EPS))) + qtv
    wnorm = (w0 / wn[None, :]).astype(np.float32)
    # reference im2col order: (dy*3+dx)*C + c -> rows (dy,c), cols (dx,f)
    wt_bf = np.ascontiguousarray(
        wnorm.reshape(3, 3, C, F).transpose(0, 2, 1, 3)
    ).astype(ml_dtypes.bfloat16).reshape(-1)

    padded = np.zeros((XP, XP, C), dtype=np.float32)
    padded[1:225, 1:225, :] = image_b
    xpT = np.zeros((C, XPN), dtype=ml_dtypes.bfloat16)
    xpT[:, : XP * XP] = (
        padded.reshape(XP * XP, C).T.astype(ml_dtypes.bfloat16)
    )
    return xpT.reshape(-1), wt_bf, float(qtv)


def _host_unpack(odev_b):
    """odev [28, 128, 512] bf16 -> sim over xp-base-p index."""
    arr = np.asarray(odev_b, dtype=np.float32)
    arr = arr.reshape(BANDS, ROUNDS, 4, F, CH)
    arr = arr.transpose(0, 2, 1, 4, 3)  # b, g, r, n, f
    return arr.reshape(BANDS * 4 * ROUNDS * CH, F)


_PMAP = None


def _pmap():
    global _PMAP
    if _PMAP is None:
        y, x = np.mgrid[0:H, 0:W]
        _PMAP = (y * XP + x).reshape(-1)
    return _PMAP


def kernel(image, w, p, q):
    global _compiled
    image = np.asarray(image)
    w = np.asarray(w, dtype=np.float32)
    p = np.asarray(p, dtype=np.float32)
    q = np.asarray(q, dtype=np.float32)

    in_maps = []
    qtv = None
    for b in range(B):
        xpb, wtb, qtv = _host_pack(image[b].astype(np.float32), w, q)
        in_maps.append({"xp": xpb, "wt": wtb})

    if _compiled is None or _compiled[0] != qtv:
        _compiled = (qtv, _build(qtv))
    nc = _compiled[1]

    global LAST_PROFILE
    res = run_bass_kernel_spmd(
        nc, in_maps, core_ids=list(range(B)), trace=TRACE
    )
    LAST_PROFILE = res
    if TRACE and res.exec_time_ns is not None:
        print(f"HW exec time: {res.exec_time_ns} ns")

    e = (p * p) / np.float32(100.0)  # per-filter exponent
    out = np.empty((B, H * W, F), dtype=np.float32)
    pm = _pmap()
    for b in range(B):
        sim = _host_unpack(res.results[b]["odev"])[pm]  # [H*W, F] fp32
        out[b] = np.sign(sim) * np.power(np.abs(sim) + np.float32(EPS), e[None, :])
    return out.reshape(B, H, W, F)
